# revision 5
# baseline (speedup 1.0000x reference)
"""NonMaxSuppression (5x5 local max, thr=0) on 8 trn2 NeuronCores.

Input : scores [8, 1, 2048, 2048] fp32 (full).
Output: [2, 2_000_000] int32 — (h, w) coords of survivors in global
        row-major order, padded with -1 (matches jnp.nonzero(size=...)).

Data-parallel: image b -> core b. Each core computes the dense survivor
mask for its image on-device; the host unshards (nonzero + concat + pad).

Device algorithm (per 256-col strip, slab layout, all fp32-exact):
  partitions = 16-row slabs (+2 halo rows each side, zero-padded);
  H pass: 5-max along cols via pair pyramid (P1 = pairwise max,
          R5 = window-5 max from P1 + one raw col), relu folded in
          via scalar_tensor_tensor (max with 0).
  V pass: same pairs trick along rows of R5 (row shifts = free-dim
          offsets in slab layout).
  mask  = (x >= max(M5, TINY)) — exact: M5 >= x always, so x >= M5
          iff x == M5; TINY (1e-38) rejects x <= 0 (scores are randn,
          |x| >> 1e-38 whenever x > 0).
"""
import sys

sys.path.insert(0, "/opt/trn_rl_repo")
import numpy as np

import concourse.bass as bass
from concourse import mybir
from concourse.bass_utils import run_bass_kernel_spmd

B, H, W = 8, 2048, 2048
NCORES = 8
MAX_KEYPOINTS = 2_000_000
TINY = 1e-38  # smallest normal fp32 territory; > 0, < any positive score

ROWS = 16          # image rows per partition (128 * 16 = 2048)
FR = ROWS + 4      # frame rows incl. 2-row halo each side
WT = 256           # strip width
FC = WT + 4        # frame cols incl. 2-col halo
NSTRIP = W // WT

f32 = mybir.dt.float32
u8 = mybir.dt.uint8


def _dram_ap(t, offset, pattern):
    return bass.AP(tensor=t, offset=offset, ap=pattern)


def _build():
    nc = bass.Bass()
    x_in = nc.declare_dram_parameter("scores", [H, W], f32, isOutput=False)
    m_out = nc.declare_dram_parameter("mask", [H, W], u8, isOutput=True)

    with (
        nc.sbuf_tensor("xb0", [128, FR, FC], f32) as xb0,
        nc.sbuf_tensor("xb1", [128, FR, FC], f32) as xb1,
        nc.sbuf_tensor("xb2", [128, FR, FC], f32) as xb2,
        nc.sbuf_tensor("p1", [128, FR, FC // 2], f32) as p1,
        nc.sbuf_tensor("tt", [128, FR, WT // 2], f32) as tt,
        nc.sbuf_tensor("r5", [128, FR, WT], f32) as r5,
        nc.sbuf_tensor("p2", [128, FR // 2, WT], f32) as p2,
        nc.sbuf_tensor("t3", [128, ROWS // 2, WT], f32) as t3,
        nc.sbuf_tensor("m5", [128, ROWS, WT], f32) as m5,
        nc.sbuf_tensor("msk", [128, ROWS, W], u8) as msk,
        nc.Block() as block,
        nc.semaphore("load_sem") as load_sem,
        nc.semaphore("init_sem") as init_sem,
        nc.semaphore("free_sem") as free_sem,
        nc.semaphore("out_sem") as out_sem,
    ):
        bufs = [xb0, xb1, xb2]
        buf_of = lambda s: bufs[2] if s == NSTRIP - 1 else bufs[s % 2]

        def strip_src(s):
            # frame col f = image col WT*s - 2 + f
            c0 = WT * s - 2
            dc = max(0, -c0)           # dst col offset
            c0 = max(0, c0)
            c1 = min(W, WT * s - 2 + FC)
            return c0, dc, c1 - c0

        @block.sync
        def _(sync):
            sync.wait_ge(init_sem, 1)
            for s in range(NSTRIP):
                xb = buf_of(s)
                if s >= 2 and s != NSTRIP - 1:
                    sync.wait_ge(free_sem, s - 1)
                c0, dc, cw = strip_src(s)
                # partitions 1..126: rows 16p-2 .. 16p+17
                sync.dma_start(
                    out=xb[1:127, :, dc : dc + cw],
                    in_=_dram_ap(
                        x_in, 14 * W + c0, [[16 * W, 126], [W, FR], [1, cw]]
                    ),
                ).then_inc(load_sem, 16)
                # partition 0: rows 0..17 -> frame rows 2..19
                sync.dma_start(
                    out=xb[0:1, 2:FR, dc : dc + cw],
                    in_=_dram_ap(x_in, c0, [[0, 1], [W, FR - 2], [1, cw]]),
                ).then_inc(load_sem, 16)
                # partition 127: rows 2030..2047 -> frame rows 0..17
                sync.dma_start(
                    out=xb[127:128, 0 : FR - 2, dc : dc + cw],
                    in_=_dram_ap(
                        x_in, 2030 * W + c0, [[0, 1], [W, FR - 2], [1, cw]]
                    ),
                ).then_inc(load_sem, 16)

        @block.vector
        def _(v):
            A = mybir.AluOpType
            # one-time zero of halo regions never written by loads
            for xb in bufs:
                v.memset(xb[0:1, 0:2, :], 0.0)              # p0 top halo rows
                # bottom halo of p127: quadrant-aligned access; p96..126's
                # copy of these rows is overwritten by every strip load
                v.memset(xb[96:128, FR - 2 : FR, :], 0.0)
            v.memset(bufs[0][:, :, 0:2], 0.0)               # strip 0 left halo
            v.memset(bufs[2][:, :, FC - 2 : FC], 0.0)       # last strip right
            v.drain().then_inc(init_sem, 1)
            for s in range(NSTRIP):
                xb = buf_of(s)
                v.wait_ge(load_sem, 48 * (s + 1))
                # --- H pass ---
                v.scalar_tensor_tensor(
                    out=p1[:, :, :], in0=xb[:, :, 0:FC:2], scalar=0.0,
                    in1=xb[:, :, 1:FC:2], op0=A.max, op1=A.max,
                )
                v.tensor_tensor(
                    out=tt[:, :, :], in0=p1[:, :, 0:128], in1=p1[:, :, 1:129],
                    op=A.max,
                )
                v.scalar_tensor_tensor(
                    out=r5[:, :, 0:WT:2], in0=xb[:, :, 4:FC:2], scalar=0.0,
                    in1=tt[:, :, :], op0=A.max, op1=A.max,
                )
                v.tensor_tensor(
                    out=tt[:, :, :], in0=p1[:, :, 1:129], in1=p1[:, :, 2:130],
                    op=A.max,
                )
                v.scalar_tensor_tensor(
                    out=r5[:, :, 1:WT:2], in0=xb[:, :, 1 : FC - 4 : 2],
                    scalar=0.0, in1=tt[:, :, :], op0=A.max, op1=A.max,
                )
                # --- V pass ---
                v.tensor_tensor(
                    out=p2[:, :, :], in0=r5[:, 0:FR:2, :], in1=r5[:, 1:FR:2, :],
                    op=A.max,
                )
                v.tensor_tensor(
                    out=t3[:, :, :], in0=p2[:, 0:8, :], in1=p2[:, 1:9, :],
                    op=A.max,
                )
                v.scalar_tensor_tensor(
                    out=m5[:, 0:ROWS:2, :], in0=r5[:, 4:FR:2, :], scalar=TINY,
                    in1=t3[:, :, :], op0=A.max, op1=A.max,
                )
                v.tensor_tensor(
                    out=t3[:, :, :], in0=p2[:, 1:9, :], in1=p2[:, 2:10, :],
                    op=A.max,
                )
                v.scalar_tensor_tensor(
                    out=m5[:, 1:ROWS:2, :], in0=r5[:, 1:ROWS:2, :], scalar=TINY,
                    in1=t3[:, :, :], op0=A.max, op1=A.max,
                )
                # --- mask ---
                v.tensor_tensor(
                    out=msk[:, :, WT * s : WT * (s + 1)],
                    in0=xb[:, 2 : 2 + ROWS, 2 : 2 + WT], in1=m5[:, :, :],
                    op=A.is_ge,
                )
                v.drain().then_inc(free_sem, 1)

        @block.scalar
        def _(sc):
            sc.wait_ge(free_sem, NSTRIP)
            sc.dma_start(
                out=_dram_ap(m_out, 0, [[16 * W, 128], [W, ROWS], [1, W]]),
                in_=msk[:, :, :],
            ).then_inc(out_sem, 16)
            sc.wait_ge(out_sem, 16)

    return nc


_nc = None


def kernel(scores: np.ndarray) -> np.ndarray:
    global _nc
    scores = np.ascontiguousarray(np.asarray(scores), dtype=np.float32)
    assert scores.shape == (B, 1, H, W), scores.shape
    if _nc is None:
        _nc = _build()
    in_maps = [
        {"scores": np.ascontiguousarray(scores[b, 0])} for b in range(NCORES)
    ]
    res = run_bass_kernel_spmd(_nc, in_maps, list(range(NCORES)), trace=False)
    hs, ws = [], []
    for b in range(NCORES):
        mask = res.results[b]["mask"]
        idx = np.flatnonzero(mask)  # row-major == (h, w) lexicographic
        hs.append((idx // W).astype(np.int32))
        ws.append((idx % W).astype(np.int32))
    hh = np.concatenate(hs)
    ww = np.concatenate(ws)
    n = min(len(hh), MAX_KEYPOINTS)
    out = np.full((2, MAX_KEYPOINTS), -1, dtype=np.int32)
    out[0, :n] = hh[:n]
    out[1, :n] = ww[:n]
    return out


if __name__ == "__main__":
    rng = np.random.default_rng(0)
    x = rng.standard_normal((B, 1, H, W), dtype=np.float32)
    out = kernel(scores=x)
    print("out", out.shape, out.dtype, "nvalid:", int((out[0] >= 0).sum()))


# revision 7
# speedup vs baseline: 1.0294x; 1.0294x over previous
"""NonMaxSuppression (5x5 local max, thr=0) on 8 trn2 NeuronCores.

Input : scores [8, 1, 2048, 2048] fp32 (full).
Output: [2, 2_000_000] int32 — (h, w) coords of survivors in global
        row-major order, padded with -1 (matches jnp.nonzero(size=...)).

Data-parallel: image b -> core b. Each core computes the dense survivor
mask for its image on-device; the host unshards (nonzero + concat + pad).

Device algorithm (per 256-col strip, slab layout, all fp32-exact):
  partitions = 16-row slabs (+2 halo rows each side, zero-padded);
  H pass: 5-max along cols via pair pyramid (P1 = pairwise max,
          R5 = window-5 max from P1 + one raw col), relu folded in
          via scalar_tensor_tensor (max with 0).
  V pass: same pairs trick along rows of R5 (row shifts = free-dim
          offsets in slab layout).
  mask  = (x >= max(M5, TINY)) — exact: M5 >= x always, so x >= M5
          iff x == M5; TINY (1e-38) rejects x <= 0 (scores are randn,
          |x| >> 1e-38 whenever x > 0).
"""
import sys

sys.path.insert(0, "/opt/trn_rl_repo")
import numpy as np

import concourse.bass as bass
from concourse import mybir
from concourse.bass_utils import run_bass_kernel_spmd

B, H, W = 8, 2048, 2048
NCORES = 8
MAX_KEYPOINTS = 2_000_000
TINY = 1e-38  # smallest normal fp32 territory; > 0, < any positive score

ROWS = 16          # image rows per partition (128 * 16 = 2048)
FR = ROWS + 4      # frame rows incl. 2-row halo each side
WT = 256           # strip width
FC = WT + 4        # frame cols incl. 2-col halo
NSTRIP = W // WT

f32 = mybir.dt.float32
u8 = mybir.dt.uint8


def _dram_ap(t, offset, pattern):
    return bass.AP(tensor=t, offset=offset, ap=pattern)


def _build():
    nc = bass.Bass()
    x_in = nc.declare_dram_parameter("scores", [H, W], f32, isOutput=False)
    m_out = nc.declare_dram_parameter("mask", [H, W], u8, isOutput=True)

    with (
        nc.sbuf_tensor("xb0", [128, FR, FC], f32) as xb0,
        nc.sbuf_tensor("xb1", [128, FR, FC], f32) as xb1,
        nc.sbuf_tensor("xb2", [128, FR, FC], f32) as xb2,
        nc.sbuf_tensor("p1", [128, FR, FC // 2], f32) as p1,
        nc.sbuf_tensor("tt", [128, FR, WT // 2], f32) as tt,
        nc.sbuf_tensor("r5", [128, FR, WT], f32) as r5,
        nc.sbuf_tensor("p2", [128, FR // 2, WT], f32) as p2,
        nc.sbuf_tensor("t3", [128, ROWS // 2, WT], f32) as t3,
        nc.sbuf_tensor("m5", [128, ROWS, WT], f32) as m5,
        nc.sbuf_tensor("msk", [128, ROWS, W], u8) as msk,
        nc.Block() as block,
        nc.semaphore("load_sem") as load_sem,
        nc.semaphore("init_sem") as init_sem,
        nc.semaphore("free_sem") as free_sem,
        nc.semaphore("out_sem") as out_sem,
    ):
        bufs = [xb0, xb1, xb2]
        buf_of = lambda s: bufs[2] if s == NSTRIP - 1 else bufs[s % 2]

        def strip_src(s):
            # frame col f = image col WT*s - 2 + f
            c0 = WT * s - 2
            dc = max(0, -c0)           # dst col offset
            c0 = max(0, c0)
            c1 = min(W, WT * s - 2 + FC)
            return c0, dc, c1 - c0

        @block.sync
        def _(sync):
            sync.wait_ge(init_sem, 1)
            for s in range(NSTRIP):
                xb = buf_of(s)
                if s >= 2 and s != NSTRIP - 1:
                    sync.wait_ge(free_sem, s - 1)
                c0, dc, cw = strip_src(s)
                # partitions 1..126: rows 16p-2 .. 16p+17
                sync.dma_start(
                    out=xb[1:127, :, dc : dc + cw],
                    in_=_dram_ap(
                        x_in, 14 * W + c0, [[16 * W, 126], [W, FR], [1, cw]]
                    ),
                ).then_inc(load_sem, 16)
                # partition 0: rows 0..17 -> frame rows 2..19
                sync.dma_start(
                    out=xb[0:1, 2:FR, dc : dc + cw],
                    in_=_dram_ap(x_in, c0, [[0, 1], [W, FR - 2], [1, cw]]),
                ).then_inc(load_sem, 16)
                # partition 127: rows 2030..2047 -> frame rows 0..17
                sync.dma_start(
                    out=xb[127:128, 0 : FR - 2, dc : dc + cw],
                    in_=_dram_ap(
                        x_in, 2030 * W + c0, [[0, 1], [W, FR - 2], [1, cw]]
                    ),
                ).then_inc(load_sem, 16)

        @block.vector
        def _(v):
            A = mybir.AluOpType
            for s in range(NSTRIP):
                xb = buf_of(s)
                v.wait_ge(load_sem, 48 * (s + 1))
                # --- H pass ---
                v.scalar_tensor_tensor(
                    out=p1[:, :, :], in0=xb[:, :, 0:FC:2], scalar=0.0,
                    in1=xb[:, :, 1:FC:2], op0=A.max, op1=A.max,
                )
                v.tensor_tensor(
                    out=tt[:, :, :], in0=p1[:, :, 0:128], in1=p1[:, :, 1:129],
                    op=A.max,
                )
                v.scalar_tensor_tensor(
                    out=r5[:, :, 0:WT:2], in0=xb[:, :, 4:FC:2], scalar=0.0,
                    in1=tt[:, :, :], op0=A.max, op1=A.max,
                )
                v.tensor_tensor(
                    out=tt[:, :, :], in0=p1[:, :, 1:129], in1=p1[:, :, 2:130],
                    op=A.max,
                )
                v.scalar_tensor_tensor(
                    out=r5[:, :, 1:WT:2], in0=xb[:, :, 1 : FC - 4 : 2],
                    scalar=0.0, in1=tt[:, :, :], op0=A.max, op1=A.max,
                )
                # --- V pass ---
                v.tensor_tensor(
                    out=p2[:, :, :], in0=r5[:, 0:FR:2, :], in1=r5[:, 1:FR:2, :],
                    op=A.max,
                )
                v.tensor_tensor(
                    out=t3[:, :, :], in0=p2[:, 0:8, :], in1=p2[:, 1:9, :],
                    op=A.max,
                )
                v.scalar_tensor_tensor(
                    out=m5[:, 0:ROWS:2, :], in0=r5[:, 4:FR:2, :], scalar=TINY,
                    in1=t3[:, :, :], op0=A.max, op1=A.max,
                )
                v.tensor_tensor(
                    out=t3[:, :, :], in0=p2[:, 1:9, :], in1=p2[:, 2:10, :],
                    op=A.max,
                )
                v.scalar_tensor_tensor(
                    out=m5[:, 1:ROWS:2, :], in0=r5[:, 1:ROWS:2, :], scalar=TINY,
                    in1=t3[:, :, :], op0=A.max, op1=A.max,
                )
                # --- mask ---
                v.tensor_tensor(
                    out=msk[:, :, WT * s : WT * (s + 1)],
                    in0=xb[:, 2 : 2 + ROWS, 2 : 2 + WT], in1=m5[:, :, :],
                    op=A.is_ge,
                )
                v.drain().then_inc(free_sem, 1)

        @block.gpsimd
        def _(g):
            # one-time zero of halo regions never written by loads; wide
            # partition ranges keep accesses quadrant-aligned — the extra
            # partitions' copies are overwritten by every strip load
            for xb in bufs:
                g.memset(xb[0:16, 0:2, :], 0.0)          # p0 top halo rows
                g.memset(xb[96:128, FR - 2 : FR, :], 0.0)  # p127 bottom halo
            g.memset(bufs[0][:, :, 0:2], 0.0)            # strip 0 left halo
            g.memset(bufs[2][:, :, FC - 2 : FC], 0.0)    # last strip right
            g.drain().then_inc(init_sem, 1)

        @block.scalar
        def _(sc):
            # stream mask strips out as compute finishes them
            for s in range(NSTRIP):
                sc.wait_ge(free_sem, s + 1)
                sc.dma_start(
                    out=_dram_ap(
                        m_out, WT * s, [[16 * W, 128], [W, ROWS], [1, WT]]
                    ),
                    in_=msk[:, :, WT * s : WT * (s + 1)],
                ).then_inc(out_sem, 16)
            sc.wait_ge(out_sem, 16 * NSTRIP)

    return nc


_nc = None


def kernel(scores: np.ndarray) -> np.ndarray:
    global _nc
    scores = np.ascontiguousarray(np.asarray(scores), dtype=np.float32)
    assert scores.shape == (B, 1, H, W), scores.shape
    if _nc is None:
        _nc = _build()
    in_maps = [
        {"scores": np.ascontiguousarray(scores[b, 0])} for b in range(NCORES)
    ]
    res = run_bass_kernel_spmd(_nc, in_maps, list(range(NCORES)), trace=False)
    hs, ws = [], []
    for b in range(NCORES):
        mask = res.results[b]["mask"]
        idx = np.flatnonzero(mask)  # row-major == (h, w) lexicographic
        hs.append((idx // W).astype(np.int32))
        ws.append((idx % W).astype(np.int32))
    hh = np.concatenate(hs)
    ww = np.concatenate(ws)
    n = min(len(hh), MAX_KEYPOINTS)
    out = np.full((2, MAX_KEYPOINTS), -1, dtype=np.int32)
    out[0, :n] = hh[:n]
    out[1, :n] = ww[:n]
    return out


if __name__ == "__main__":
    rng = np.random.default_rng(0)
    x = rng.standard_normal((B, 1, H, W), dtype=np.float32)
    out = kernel(scores=x)
    print("out", out.shape, out.dtype, "nvalid:", int((out[0] >= 0).sum()))


# revision 8
# speedup vs baseline: 1.1965x; 1.1624x over previous
"""NonMaxSuppression (5x5 local max, thr=0) on 8 trn2 NeuronCores.

Input : scores [8, 1, 2048, 2048] fp32 (full).
Output: [2, 2_000_000] int32 — (h, w) coords of survivors in global
        row-major order, padded with -1 (matches jnp.nonzero(size=...)).

Data-parallel: image b -> core b. Each core computes the dense survivor
mask for its image on-device; the host unshards (nonzero + concat + pad).

Device algorithm (per 256-col strip, slab layout, all fp32-exact):
  partitions = 16-row slabs (+2 halo rows each side, zero-padded);
  H pass: 5-max along cols via pair pyramid (P1 = pairwise max,
          R5 = window-5 max from P1 + one raw col), relu folded in
          via scalar_tensor_tensor (max with 0).
  V pass: same pairs trick along rows of R5 (row shifts = free-dim
          offsets in slab layout).
  mask  = (x >= max(M5, TINY)) — exact: M5 >= x always, so x >= M5
          iff x == M5; TINY (1e-38) rejects x <= 0 (scores are randn,
          |x| >> 1e-38 whenever x > 0).
"""
import sys

sys.path.insert(0, "/opt/trn_rl_repo")
import numpy as np

import concourse.bass as bass
from concourse import mybir
from concourse.bass_utils import run_bass_kernel_spmd

B, H, W = 8, 2048, 2048
NCORES = 8
MAX_KEYPOINTS = 2_000_000
TINY = 1e-38  # smallest normal fp32 territory; > 0, < any positive score

ROWS = 16          # image rows per partition (128 * 16 = 2048)
FR = ROWS + 4      # frame rows incl. 2-row halo each side
WT = 256           # strip width
FC = WT + 4        # frame cols incl. 2-col halo
NSTRIP = W // WT

f32 = mybir.dt.float32
u8 = mybir.dt.uint8


def _dram_ap(t, offset, pattern):
    return bass.AP(tensor=t, offset=offset, ap=pattern)


def _build():
    nc = bass.Bass()
    x_in = nc.declare_dram_parameter("scores", [H, W], f32, isOutput=False)
    m_out = nc.declare_dram_parameter("mask", [H, W], u8, isOutput=True)

    with (
        nc.sbuf_tensor("xb0", [128, FR, FC], f32) as xb0,
        nc.sbuf_tensor("xb1", [128, FR, FC], f32) as xb1,
        nc.sbuf_tensor("xb2", [128, FR, FC], f32) as xb2,
        nc.sbuf_tensor("p1", [128, FR, FC // 2], f32) as p1,
        nc.sbuf_tensor("tt", [128, FR, WT // 2 + 1], f32) as tt,
        nc.sbuf_tensor("r5", [128, FR, WT], f32) as r5,
        nc.sbuf_tensor("p2", [128, FR // 2, WT], f32) as p2,
        nc.sbuf_tensor("t3", [128, ROWS // 2 + 1, WT], f32) as t3,
        nc.sbuf_tensor("m5", [128, ROWS, WT], f32) as m5,
        nc.sbuf_tensor("msk", [128, ROWS, W], u8) as msk,
        nc.Block() as block,
        nc.semaphore("load_sem") as load_sem,
        nc.semaphore("init_sem") as init_sem,
        nc.semaphore("free_sem") as free_sem,
        nc.semaphore("out_sem") as out_sem,
    ):
        bufs = [xb0, xb1, xb2]
        buf_of = lambda s: bufs[2] if s == NSTRIP - 1 else bufs[s % 2]

        def strip_src(s):
            # frame col f = image col WT*s - 2 + f
            c0 = WT * s - 2
            dc = max(0, -c0)           # dst col offset
            c0 = max(0, c0)
            c1 = min(W, WT * s - 2 + FC)
            return c0, dc, c1 - c0

        @block.sync
        def _(sync):
            sync.wait_ge(init_sem, 1)
            for s in range(NSTRIP):
                xb = buf_of(s)
                if s >= 2 and s != NSTRIP - 1:
                    sync.wait_ge(free_sem, s - 1)
                c0, dc, cw = strip_src(s)
                # partitions 1..126: rows 16p-2 .. 16p+17
                sync.dma_start(
                    out=xb[1:127, :, dc : dc + cw],
                    in_=_dram_ap(
                        x_in, 14 * W + c0, [[16 * W, 126], [W, FR], [1, cw]]
                    ),
                ).then_inc(load_sem, 16)
                # partition 0: rows 0..17 -> frame rows 2..19
                sync.dma_start(
                    out=xb[0:1, 2:FR, dc : dc + cw],
                    in_=_dram_ap(x_in, c0, [[0, 1], [W, FR - 2], [1, cw]]),
                ).then_inc(load_sem, 16)
                # partition 127: rows 2030..2047 -> frame rows 0..17
                sync.dma_start(
                    out=xb[127:128, 0 : FR - 2, dc : dc + cw],
                    in_=_dram_ap(
                        x_in, 2030 * W + c0, [[0, 1], [W, FR - 2], [1, cw]]
                    ),
                ).then_inc(load_sem, 16)

        @block.vector
        def _(v):
            A = mybir.AluOpType
            for s in range(NSTRIP):
                xb = buf_of(s)
                v.wait_ge(load_sem, 48 * (s + 1))
                # --- H pass ---
                v.scalar_tensor_tensor(
                    out=p1[:, :, :], in0=xb[:, :, 0:FC:2], scalar=0.0,
                    in1=xb[:, :, 1:FC:2], op0=A.max, op1=A.max,
                )
                # Tall[k] = max(P1[k], P1[k+1]); Te = Tall[:-1], To = Tall[1:]
                v.tensor_tensor(
                    out=tt[:, :, :], in0=p1[:, :, 0:129], in1=p1[:, :, 1:130],
                    op=A.max,
                )
                v.scalar_tensor_tensor(
                    out=r5[:, :, 0:WT:2], in0=xb[:, :, 4:FC:2], scalar=0.0,
                    in1=tt[:, :, 0:128], op0=A.max, op1=A.max,
                )
                v.scalar_tensor_tensor(
                    out=r5[:, :, 1:WT:2], in0=xb[:, :, 1 : FC - 4 : 2],
                    scalar=0.0, in1=tt[:, :, 1:129], op0=A.max, op1=A.max,
                )
                # --- V pass ---
                v.tensor_tensor(
                    out=p2[:, :, :], in0=r5[:, 0:FR:2, :], in1=r5[:, 1:FR:2, :],
                    op=A.max,
                )
                # T3[i] = max(P2[i], P2[i+1]); even rows use T3[:-1], odd T3[1:]
                v.tensor_tensor(
                    out=t3[:, :, :], in0=p2[:, 0:9, :], in1=p2[:, 1:10, :],
                    op=A.max,
                )
                v.scalar_tensor_tensor(
                    out=m5[:, 0:ROWS:2, :], in0=r5[:, 4:FR:2, :], scalar=TINY,
                    in1=t3[:, 0:8, :], op0=A.max, op1=A.max,
                )
                v.scalar_tensor_tensor(
                    out=m5[:, 1:ROWS:2, :], in0=r5[:, 1:ROWS:2, :], scalar=TINY,
                    in1=t3[:, 1:9, :], op0=A.max, op1=A.max,
                )
                # --- mask ---
                v.tensor_tensor(
                    out=msk[:, :, WT * s : WT * (s + 1)],
                    in0=xb[:, 2 : 2 + ROWS, 2 : 2 + WT], in1=m5[:, :, :],
                    op=A.is_ge,
                )
                v.drain().then_inc(free_sem, 1)

        @block.gpsimd
        def _(g):
            # one-time zero of halo regions never written by loads; wide
            # partition ranges keep accesses quadrant-aligned — the extra
            # partitions' copies are overwritten by every strip load
            for xb in bufs:
                g.memset(xb[0:16, 0:2, :], 0.0)          # p0 top halo rows
                g.memset(xb[96:128, FR - 2 : FR, :], 0.0)  # p127 bottom halo
            g.memset(bufs[0][:, :, 0:2], 0.0)            # strip 0 left halo
            g.memset(bufs[2][:, :, FC - 2 : FC], 0.0)    # last strip right
            g.drain().then_inc(init_sem, 1)

        @block.scalar
        def _(sc):
            # stream mask strips out as compute finishes them
            for s in range(NSTRIP):
                sc.wait_ge(free_sem, s + 1)
                sc.dma_start(
                    out=_dram_ap(
                        m_out, WT * s, [[16 * W, 128], [W, ROWS], [1, WT]]
                    ),
                    in_=msk[:, :, WT * s : WT * (s + 1)],
                ).then_inc(out_sem, 16)
            sc.wait_ge(out_sem, 16 * NSTRIP)

    return nc


_nc = None


def kernel(scores: np.ndarray) -> np.ndarray:
    global _nc
    scores = np.ascontiguousarray(np.asarray(scores), dtype=np.float32)
    assert scores.shape == (B, 1, H, W), scores.shape
    if _nc is None:
        _nc = _build()
    in_maps = [
        {"scores": np.ascontiguousarray(scores[b, 0])} for b in range(NCORES)
    ]
    res = run_bass_kernel_spmd(_nc, in_maps, list(range(NCORES)), trace=False)
    hs, ws = [], []
    for b in range(NCORES):
        mask = res.results[b]["mask"]
        idx = np.flatnonzero(mask)  # row-major == (h, w) lexicographic
        hs.append((idx // W).astype(np.int32))
        ws.append((idx % W).astype(np.int32))
    hh = np.concatenate(hs)
    ww = np.concatenate(ws)
    n = min(len(hh), MAX_KEYPOINTS)
    out = np.full((2, MAX_KEYPOINTS), -1, dtype=np.int32)
    out[0, :n] = hh[:n]
    out[1, :n] = ww[:n]
    return out


if __name__ == "__main__":
    rng = np.random.default_rng(0)
    x = rng.standard_normal((B, 1, H, W), dtype=np.float32)
    out = kernel(scores=x)
    print("out", out.shape, out.dtype, "nvalid:", int((out[0] >= 0).sum()))


# revision 10
# speedup vs baseline: 1.1966x; 1.0001x over previous
"""NonMaxSuppression (5x5 local max, thr=0) on 8 trn2 NeuronCores.

Input : scores [8, 1, 2048, 2048] fp32 (full).
Output: [2, 2_000_000] int32 — (h, w) coords of survivors in global
        row-major order, padded with -1 (matches jnp.nonzero(size=...)).

Data-parallel: image b -> core b. Each core computes the dense survivor
mask for its image on-device; the host unshards (nonzero + concat + pad).

Device algorithm (per 256-col strip, slab layout, all fp32-exact):
  partitions = 16-row slabs (+2 halo rows each side, zero-padded);
  H pass: 5-max along cols via pair pyramid (P1 = pairwise max,
          R5 = window-5 max from P1 + one raw col), relu folded in
          via scalar_tensor_tensor (max with 0).
  V pass: same pairs trick along rows of R5 (row shifts = free-dim
          offsets in slab layout).
  mask  = (x >= max(M5, TINY)) — exact: M5 >= x always, so x >= M5
          iff x == M5; TINY (1e-38) rejects x <= 0 (scores are randn,
          |x| >> 1e-38 whenever x > 0).
"""
import sys

sys.path.insert(0, "/opt/trn_rl_repo")
import numpy as np

import concourse.bass as bass
from concourse import mybir
from concourse.bass_utils import run_bass_kernel_spmd

B, H, W = 8, 2048, 2048
NCORES = 8
MAX_KEYPOINTS = 2_000_000
TINY = 1e-38  # smallest normal fp32 territory; > 0, < any positive score

ROWS = 16          # image rows per partition (128 * 16 = 2048)
FR = ROWS + 4      # frame rows incl. 2-row halo each side
WT = 256           # strip width
FC = WT + 4        # frame cols incl. 2-col halo
NSTRIP = W // WT

f32 = mybir.dt.float32
u8 = mybir.dt.uint8


def _dram_ap(t, offset, pattern):
    return bass.AP(tensor=t, offset=offset, ap=pattern)


def _build():
    nc = bass.Bass()
    x_in = nc.declare_dram_parameter("scores", [H, W], f32, isOutput=False)
    m_out = nc.declare_dram_parameter("mask", [H, W], u8, isOutput=True)

    with (
        nc.sbuf_tensor("xb0", [128, FR, FC], f32) as xb0,
        nc.sbuf_tensor("xb1", [128, FR, FC], f32) as xb1,
        nc.sbuf_tensor("xb2", [128, FR, FC], f32) as xb2,
        nc.sbuf_tensor("p1", [128, FR, FC // 2], f32) as p1,
        nc.sbuf_tensor("tt", [128, FR, WT // 2 + 1], f32) as tt,
        nc.sbuf_tensor("r5", [128, FR, WT], f32) as r5,
        nc.sbuf_tensor("p2", [128, FR // 2, WT], f32) as p2,
        nc.sbuf_tensor("t3", [128, ROWS // 2 + 1, WT], f32) as t3,
        nc.sbuf_tensor("m5", [128, ROWS, WT], f32) as m5,
        nc.sbuf_tensor("msk", [128, ROWS, W], u8) as msk,
        nc.Block() as block,
        nc.semaphore("load_sem") as load_sem,
        nc.semaphore("init_sem") as init_sem,
        nc.semaphore("free_sem") as free_sem,
        nc.semaphore("out_sem") as out_sem,
    ):
        bufs = [xb0, xb1, xb2]
        buf_of = lambda s: bufs[2] if s == NSTRIP - 1 else bufs[s % 2]

        def strip_src(s):
            # frame col f = image col WT*s - 2 + f
            c0 = WT * s - 2
            dc = max(0, -c0)           # dst col offset
            c0 = max(0, c0)
            c1 = min(W, WT * s - 2 + FC)
            return c0, dc, c1 - c0

        @block.sync
        def _(sync):
            sync.wait_ge(init_sem, 1)
            for s in range(NSTRIP):
                xb = buf_of(s)
                if s >= 2 and s != NSTRIP - 1:
                    sync.wait_ge(free_sem, s - 1)
                c0, dc, cw = strip_src(s)
                # partitions 1..126: rows 16p-2 .. 16p+17
                sync.dma_start(
                    out=xb[1:127, :, dc : dc + cw],
                    in_=_dram_ap(
                        x_in, 14 * W + c0, [[16 * W, 126], [W, FR], [1, cw]]
                    ),
                ).then_inc(load_sem, 16)
                # partition 0: rows 0..17 -> frame rows 2..19
                sync.dma_start(
                    out=xb[0:1, 2:FR, dc : dc + cw],
                    in_=_dram_ap(x_in, c0, [[0, 1], [W, FR - 2], [1, cw]]),
                ).then_inc(load_sem, 16)
                # partition 127: rows 2030..2047 -> frame rows 0..17
                sync.dma_start(
                    out=xb[127:128, 0 : FR - 2, dc : dc + cw],
                    in_=_dram_ap(
                        x_in, 2030 * W + c0, [[0, 1], [W, FR - 2], [1, cw]]
                    ),
                ).then_inc(load_sem, 16)

        @block.vector
        def _(v):
            A = mybir.AluOpType
            for s in range(NSTRIP):
                xb = buf_of(s)
                v.wait_ge(load_sem, 48 * (s + 1))
                # --- H pass ---
                v.scalar_tensor_tensor(
                    out=p1[:, :, :], in0=xb[:, :, 0:FC:2], scalar=0.0,
                    in1=xb[:, :, 1:FC:2], op0=A.max, op1=A.max,
                )
                # Tall[k] = max(P1[k], P1[k+1]); Te = Tall[:-1], To = Tall[1:]
                v.tensor_tensor(
                    out=tt[:, :, :], in0=p1[:, :, 0:129], in1=p1[:, :, 1:130],
                    op=A.max,
                )
                v.scalar_tensor_tensor(
                    out=r5[:, :, 0:WT:2], in0=xb[:, :, 4:FC:2], scalar=0.0,
                    in1=tt[:, :, 0:128], op0=A.max, op1=A.max,
                )
                v.scalar_tensor_tensor(
                    out=r5[:, :, 1:WT:2], in0=xb[:, :, 1 : FC - 4 : 2],
                    scalar=0.0, in1=tt[:, :, 1:129], op0=A.max, op1=A.max,
                )
                # --- V pass ---
                v.tensor_tensor(
                    out=p2[:, :, :], in0=r5[:, 0:FR:2, :], in1=r5[:, 1:FR:2, :],
                    op=A.max,
                )
                # T3[i] = max(P2[i], P2[i+1]); even rows use T3[:-1], odd T3[1:]
                v.tensor_tensor(
                    out=t3[:, :, :], in0=p2[:, 0:9, :], in1=p2[:, 1:10, :],
                    op=A.max,
                )
                v.scalar_tensor_tensor(
                    out=m5[:, 0:ROWS:2, :], in0=r5[:, 4:FR:2, :], scalar=TINY,
                    in1=t3[:, 0:8, :], op0=A.max, op1=A.max,
                )
                v.scalar_tensor_tensor(
                    out=m5[:, 1:ROWS:2, :], in0=r5[:, 1:ROWS:2, :], scalar=TINY,
                    in1=t3[:, 1:9, :], op0=A.max, op1=A.max,
                )
                # --- mask ---
                v.tensor_tensor(
                    out=msk[:, :, WT * s : WT * (s + 1)],
                    in0=xb[:, 2 : 2 + ROWS, 2 : 2 + WT], in1=m5[:, :, :],
                    op=A.is_ge,
                )
                v.drain().then_inc(free_sem, 1)

        @block.gpsimd
        def _(g):
            # one-time zero of halo regions never written by loads; wide
            # partition ranges keep accesses quadrant-aligned — the extra
            # partitions' copies are overwritten by every strip load
            for xb in bufs:
                g.memset(xb[0:16, 0:2, :], 0.0)          # p0 top halo rows
                g.memset(xb[96:128, FR - 2 : FR, :], 0.0)  # p127 bottom halo
            g.memset(bufs[0][:, :, 0:2], 0.0)            # strip 0 left halo
            g.memset(bufs[2][:, :, FC - 2 : FC], 0.0)    # last strip right
            g.drain().then_inc(init_sem, 1)

        @block.scalar
        def _(sc):
            # stream mask strips out as compute finishes them
            for s in range(NSTRIP):
                sc.wait_ge(free_sem, s + 1)
                sc.dma_start(
                    out=_dram_ap(
                        m_out, WT * s, [[16 * W, 128], [W, ROWS], [1, WT]]
                    ),
                    in_=msk[:, :, WT * s : WT * (s + 1)],
                ).then_inc(out_sem, 16)
            sc.wait_ge(out_sem, 16 * NSTRIP)

    return nc


_nc = None


def kernel(scores: np.ndarray) -> np.ndarray:
    global _nc
    scores = np.ascontiguousarray(np.asarray(scores), dtype=np.float32)
    assert scores.shape == (B, 1, H, W), scores.shape
    if _nc is None:
        _nc = _build()
    in_maps = [
        {"scores": np.ascontiguousarray(scores[b, 0])} for b in range(NCORES)
    ]
    res = run_bass_kernel_spmd(_nc, in_maps, list(range(NCORES)), trace=False)
    hs, ws = [], []
    for b in range(NCORES):
        mask = res.results[b]["mask"]
        idx = np.flatnonzero(mask)  # row-major == (h, w) lexicographic
        hs.append((idx // W).astype(np.int32))
        ws.append((idx % W).astype(np.int32))
    hh = np.concatenate(hs)
    ww = np.concatenate(ws)
    n = min(len(hh), MAX_KEYPOINTS)
    out = np.full((2, MAX_KEYPOINTS), -1, dtype=np.int32)
    out[0, :n] = hh[:n]
    out[1, :n] = ww[:n]
    return out


if __name__ == "__main__":
    rng = np.random.default_rng(0)
    x = rng.standard_normal((B, 1, H, W), dtype=np.float32)
    out = kernel(scores=x)
    print("out", out.shape, out.dtype, "nvalid:", int((out[0] >= 0).sum()))
